# revision 9
# baseline (speedup 1.0000x reference)
"""ButterflyLinear Trainium2 kernel.

Math: out[b, s, i] = (sum_o x[b, s, o] * W[o, i]) * mask[s, i], with
mask[s, i] = 1 iff 4s <= i < 4s+4 (stride-4 band). The band makes the
output block-diagonal: s-rows [128t, 128t+128) only touch output columns
[512t, 512t+512) -- an 8x compute reduction vs the full matmul.

Sharding (8 cores): core t owns s-block t for all 16 batches
(tensor-parallel split of W columns; no inter-core communication).

Key packing trick: a 64-row s-sub-block only spans a 256-wide band
window, and that window is the same for every batch. So the matmul
stationary packs TWO batches on the partition axis (M = 128 = 2 batches
x 64 s-rows) against one N=256 W window -- W streams once per batch
PAIR, halving PE row traffic, and each accumulation lives in HALF a
PSUM bank. All 16 chains (8 batch-pairs x 2 sub-blocks) fit in the 8
banks at once: a single wave, no bank-recycling serialization.
PSUM has_written is per-element: only the first matmul of each bank
uses start=True (clears the whole bank), its half-bank partner starts
with start=False and overwrites its untouched half.

Per-core device program:
  - f32 warm-up matmuls during the DMA ramp (HAM clock-gate release)
  - 8 o-chunks streamed (W chunk 256KB + x chunk 1MB each), fp32r
  - 16 chains x 8 chunk-matmuls (N=256) accumulating in half-banks
  - 8 full-bank copies (alternating VectorE/ScalarE) -> DMA raw blocks
Host extracts the 4-wide diagonal band from the raw blocks into the
zero-filled (16, 1024, 4096) result.
"""

import os
import sys
from contextlib import ExitStack

import numpy as np

if "/opt/trn_rl_repo" not in sys.path:
    sys.path.insert(0, "/opt/trn_rl_repo")

import concourse.bass as bass  # noqa: E402
import concourse.tile as tile  # noqa: E402
from concourse import bacc, mybir  # noqa: E402
from concourse.bass_utils import run_bass_kernel_spmd  # noqa: E402

B = 16  # batch
NT = 8  # s-blocks == cores
SB = 128  # s rows per block
NC_ = 8  # o chunks
KC = 128  # o rows per chunk
NI = 512  # output columns per block
NG = 8  # batch pairs
NH = 2  # 64-row s-sub-blocks per s-block
NW = 256  # W window per sub-block
N_WARMUP = int(os.environ.get("BFK_WARMUP", "6"))  # HAM warm-up matmuls

# Matmul input dtype. fp16 (11-bit mantissa) halves DMA traffic and
# streams the PE at 1 cycle/row; measured accuracy is on par with fp32r
# (~1e-4 band rel err) since accumulation stays fp32 in PSUM.
_DT_CHOICES = {
    "f16": mybir.dt.float16,
    "f32r": mybir.dt.float32r,
    "f32": mybir.dt.float32,
}
MM_DT = _DT_CHOICES[os.environ.get("BFK_DT", "f16")]
F32 = mybir.dt.float32

_STATE: dict = {}


def _build():
    if "nc" in _STATE:
        return _STATE["nc"]

    nc = bacc.Bacc(
        "TRN2", target_bir_lowering=False, debug=False, num_devices=NT
    )
    xt = nc.dram_tensor(
        "xt", [NC_, KC, NG, NH, SB], MM_DT, kind="ExternalInput"
    ).ap()
    wt = nc.dram_tensor("wt", [NC_, KC, NH, NW], MM_DT, kind="ExternalInput").ap()
    out = nc.dram_tensor("out", [NG, SB, NI], F32, kind="ExternalOutput").ap()

    with tile.TileContext(nc) as tc, ExitStack() as ctx:
        wp = ctx.enter_context(tc.tile_pool(name="w", bufs=1))
        xp = ctx.enter_context(tc.tile_pool(name="x", bufs=1))
        pp = ctx.enter_context(tc.tile_pool(name="ps", bufs=8, space="PSUM"))
        op = ctx.enter_context(tc.tile_pool(name="o", bufs=6))
        sp = ctx.enter_context(tc.tile_pool(name="scratch", bufs=1))

        # PE warm-up: matmuls on a memset scratch tile, no DMA deps, so
        # they run during the input-DMA ramp and release the HAM clock
        # gate before the real stream starts. Must use the SAME dtype as
        # the main matmuls: mixing f32 warm-ups with f16/f32r streams
        # wedged the exec unit (FP32HI <-> FWL transition hazard).
        if N_WARMUP:
            wmt = sp.tile([KC, NI], MM_DT, tag="warm")
            nc.gpsimd.memset(wmt[:], 0.0)
            pwarm = pp.tile([SB, NI], F32, tag="ps", name="ps_warm")
            for _ in range(N_WARMUP):
                nc.tensor.matmul(
                    pwarm[:], wmt[:, :KC], wmt[:], start=True, stop=True
                )

        w_t = []
        x_t = []
        for c in range(NC_):
            w = wp.tile([KC, NH, NW], MM_DT, tag=f"w{c}")
            nc.sync.dma_start(out=w[:], in_=wt[c])
            w_t.append(w)
            xc = xp.tile([KC, NG, NH, SB], MM_DT, tag=f"x{c}")
            nc.sync.dma_start(out=xc[:], in_=xt[c])
            x_t.append(xc)

        ps = [pp.tile([SB, NI], F32, tag="ps", name=f"ps_{g}") for g in range(NG)]
        for c in range(NC_):
            for g in range(NG):
                for h in range(NH):
                    nc.tensor.matmul(
                        ps[g][:, h * NW : (h + 1) * NW],
                        x_t[c][:, g, h, :],
                        w_t[c][:, h, :],
                        start=(c == 0 and h == 0),
                        stop=(c == NC_ - 1 and h == NH - 1),
                    )
        for g in range(NG):
            ot = op.tile([SB, NI], F32, tag="ot")
            # Alternate evacuation between VectorE and ScalarE so two
            # banks drain at a time.
            if g % 2 == 1:
                nc.scalar.copy(ot[:], ps[g][:])
            else:
                nc.vector.tensor_copy(ot[:], ps[g][:])
            nc.sync.dma_start(out=out[g], in_=ot[:])

    nc.compile()
    _STATE["nc"] = nc
    return nc


def _shard(x, W):
    np_dt = mybir.dt.np(MM_DT)
    x = np.ascontiguousarray(np.asarray(x, dtype=np.float32)).astype(np_dt)
    W = np.ascontiguousarray(np.asarray(W, dtype=np.float32)).astype(np_dt)
    # xt[t][c, p, g, h, m] = x[2g + m//64, 128t + 64h + (m%64), 128c + p]
    xr = x.reshape(NG, 2, NT, NH, 64, NC_, KC)  # [g, bi, t, h, r, c, p]
    xts = np.ascontiguousarray(np.transpose(xr, (2, 5, 6, 0, 3, 1, 4))).reshape(
        NT, NC_, KC, NG, NH, SB
    )
    # wt[t][c, p, h, n] = W[128c + p, 512t + 256h + n]
    wr = W.reshape(NC_, KC, NT, NH, NW)  # [c, p, t, h, n]
    wts = np.ascontiguousarray(np.transpose(wr, (2, 0, 1, 3, 4)))
    return [{"xt": xts[t], "wt": wts[t]} for t in range(NT)]


def kernel(x, W, _trace=False, _trace_kwargs=None):
    nc = _build()
    in_maps = _shard(x, W)
    res = run_bass_kernel_spmd(
        nc,
        in_maps,
        list(range(NT)),
        trace=_trace,
        **(_trace_kwargs or {}),
    )
    _STATE["last_run"] = res
    # Band extraction: block row m = 64*bi + r holds batch 2g+bi, s-row
    # 128t + 64h + r; band value j sits at block col 256h + 4r + j.
    band = np.empty((B, NT * SB, 4), dtype=np.float32)
    for t in range(NT):
        blk = np.ascontiguousarray(res.results[t]["out"])  # (NG, 128, 512)
        e = blk.strides[2]
        v = np.lib.stride_tricks.as_strided(
            blk,
            shape=(NG, 2, NH, 64, 4),
            strides=(
                blk.strides[0],
                64 * blk.strides[1],
                NW * e,
                blk.strides[1] + 4 * e,
                e,
            ),
        )
        # [g, bi, h, r, j] -> b = 2g + bi, s_rel = 64h + r
        band[:, t * SB : (t + 1) * SB, :] = v.reshape(B, SB, 4)
    s_idx = np.arange(NT * SB)
    y = np.zeros((B, NT * SB, NT * NI), dtype=np.float32)
    y4 = y.reshape(B, NT * SB, NT * SB, 4)
    y4[:, s_idx, s_idx, :] = band
    return y


# revision 10
# speedup vs baseline: 1.1556x; 1.1556x over previous
"""ButterflyLinear Trainium2 kernel.

Math: out[b, s, i] = (sum_o x[b, s, o] * W[o, i]) * mask[s, i], with
mask[s, i] = 1 iff 4s <= i < 4s+4 (stride-4 band). The band makes the
output block-diagonal: s-rows [128t, 128t+128) only touch output columns
[512t, 512t+512) -- an 8x compute reduction vs the full matmul.

Sharding (8 cores): core t owns s-block t for all 16 batches
(tensor-parallel split of W columns; no inter-core communication).

Key packing trick: a 64-row s-sub-block only spans a 256-wide band
window, and that window is the same for every batch. So the matmul
stationary packs TWO batches on the partition axis (M = 128 = 2 batches
x 64 s-rows) against one N=256 W window -- W streams once per batch
PAIR, halving PE row traffic, and each accumulation lives in HALF a
PSUM bank. All 16 chains (8 batch-pairs x 2 sub-blocks) fit in the 8
banks at once: a single wave, no bank-recycling serialization.
PSUM has_written is per-element: only the first matmul of each bank
uses start=True (clears the whole bank), its half-bank partner starts
with start=False and overwrites its untouched half.

Per-core device program:
  - f32 warm-up matmuls during the DMA ramp (HAM clock-gate release)
  - 8 o-chunks streamed (W chunk 256KB + x chunk 1MB each), fp32r
  - 16 chains x 8 chunk-matmuls (N=256) accumulating in half-banks
  - 8 full-bank copies (alternating VectorE/ScalarE) -> DMA raw blocks
Host extracts the 4-wide diagonal band from the raw blocks into the
zero-filled (16, 1024, 4096) result.
"""

import os
import sys
from contextlib import ExitStack

import numpy as np

if "/opt/trn_rl_repo" not in sys.path:
    sys.path.insert(0, "/opt/trn_rl_repo")

import concourse.bass as bass  # noqa: E402
import concourse.tile as tile  # noqa: E402
from concourse import bacc, mybir  # noqa: E402
from concourse.bass_utils import run_bass_kernel_spmd  # noqa: E402

B = 16  # batch
NT = 8  # s-blocks == cores
SB = 128  # s rows per block
NC_ = 8  # o chunks
KC = 128  # o rows per chunk
NI = 512  # output columns per block
QB = int(os.environ.get("BFK_QB", "4"))  # batches packed per stationary
RW = SB // QB  # s-rows per sub-block
NH = QB  # sub-blocks per s-block (RW*NH = 128)
NW = 4 * RW  # W window per sub-block
NG = B // QB  # batch groups
N_WARMUP = int(os.environ.get("BFK_WARMUP", "0"))  # HAM warm-up matmuls

# Matmul input dtype. fp16 (11-bit mantissa) halves DMA traffic and
# streams the PE at 1 cycle/row; measured accuracy is on par with fp32r
# (~1e-4 band rel err) since accumulation stays fp32 in PSUM.
_DT_CHOICES = {
    "f16": mybir.dt.float16,
    "f32r": mybir.dt.float32r,
    "f32": mybir.dt.float32,
}
MM_DT = _DT_CHOICES[os.environ.get("BFK_DT", "f16")]
F32 = mybir.dt.float32

_STATE: dict = {}


def _build():
    if "nc" in _STATE:
        return _STATE["nc"]

    nc = bacc.Bacc(
        "TRN2", target_bir_lowering=False, debug=False, num_devices=NT
    )
    xt = nc.dram_tensor(
        "xt", [NC_, KC, NG, NH, SB], MM_DT, kind="ExternalInput"
    ).ap()
    wt = nc.dram_tensor("wt", [NC_, KC, NH, NW], MM_DT, kind="ExternalInput").ap()
    out = nc.dram_tensor("out", [NG, SB, NI], F32, kind="ExternalOutput").ap()

    with tile.TileContext(nc) as tc, ExitStack() as ctx:
        wp = ctx.enter_context(tc.tile_pool(name="w", bufs=1))
        xp = ctx.enter_context(tc.tile_pool(name="x", bufs=1))
        pp = ctx.enter_context(tc.tile_pool(name="ps", bufs=8, space="PSUM"))
        op = ctx.enter_context(tc.tile_pool(name="o", bufs=6))
        sp = ctx.enter_context(tc.tile_pool(name="scratch", bufs=1))

        # PE warm-up: matmuls on a memset scratch tile, no DMA deps, so
        # they run during the input-DMA ramp and release the HAM clock
        # gate before the real stream starts. Must use the SAME dtype as
        # the main matmuls: mixing f32 warm-ups with f16/f32r streams
        # wedged the exec unit (FP32HI <-> FWL transition hazard).
        if N_WARMUP:
            wmt = sp.tile([KC, NI], MM_DT, tag="warm")
            nc.gpsimd.memset(wmt[:], 0.0)
            pwarm = pp.tile([SB, NI], F32, tag="ps", name="ps_warm")
            for _ in range(N_WARMUP):
                nc.tensor.matmul(
                    pwarm[:], wmt[:, :KC], wmt[:], start=True, stop=True
                )

        w_t = []
        x_t = []
        for c in range(NC_):
            w = wp.tile([KC, NH, NW], MM_DT, tag=f"w{c}")
            nc.sync.dma_start(out=w[:], in_=wt[c])
            w_t.append(w)
            xc = xp.tile([KC, NG, NH, SB], MM_DT, tag=f"x{c}")
            nc.sync.dma_start(out=xc[:], in_=xt[c])
            x_t.append(xc)

        ps = [pp.tile([SB, NI], F32, tag="ps", name=f"ps_{g}") for g in range(NG)]
        for c in range(NC_):
            for g in range(NG):
                for h in range(NH):
                    nc.tensor.matmul(
                        ps[g][:, h * NW : (h + 1) * NW],
                        x_t[c][:, g, h, :],
                        w_t[c][:, h, :],
                        start=(c == 0 and h == 0),
                        stop=(c == NC_ - 1 and h == NH - 1),
                    )
        for g in range(NG):
            ot = op.tile([SB, NI], F32, tag="ot")
            # Alternate evacuation between VectorE and ScalarE so two
            # banks drain at a time.
            if g % 2 == 1:
                nc.scalar.copy(ot[:], ps[g][:])
            else:
                nc.vector.tensor_copy(ot[:], ps[g][:])
            nc.sync.dma_start(out=out[g], in_=ot[:])

    nc.compile()
    _STATE["nc"] = nc
    return nc


def _shard(x, W):
    np_dt = mybir.dt.np(MM_DT)
    x = np.ascontiguousarray(np.asarray(x, dtype=np.float32)).astype(np_dt)
    W = np.ascontiguousarray(np.asarray(W, dtype=np.float32)).astype(np_dt)
    # xt[t][c, p, g, h, m] = x[QB*g + m//RW, 128t + RW*h + (m%RW), 128c + p]
    xr = x.reshape(NG, QB, NT, NH, RW, NC_, KC)  # [g, qi, t, h, r, c, p]
    xts = np.ascontiguousarray(np.transpose(xr, (2, 5, 6, 0, 3, 1, 4))).reshape(
        NT, NC_, KC, NG, NH, SB
    )
    # wt[t][c, p, h, n] = W[128c + p, 512t + NW*h + n]
    wr = W.reshape(NC_, KC, NT, NH, NW)  # [c, p, t, h, n]
    wts = np.ascontiguousarray(np.transpose(wr, (2, 0, 1, 3, 4)))
    return [{"xt": xts[t], "wt": wts[t]} for t in range(NT)]


def kernel(x, W, _trace=False, _trace_kwargs=None):
    nc = _build()
    in_maps = _shard(x, W)
    res = run_bass_kernel_spmd(
        nc,
        in_maps,
        list(range(NT)),
        trace=_trace,
        **(_trace_kwargs or {}),
    )
    _STATE["last_run"] = res
    # Band extraction: block row m = RW*qi + r holds batch QB*g + qi,
    # s-row 128t + RW*h + r; band value j sits at block col NW*h + 4r + j.
    band = np.empty((B, NT * SB, 4), dtype=np.float32)
    for t in range(NT):
        blk = np.ascontiguousarray(res.results[t]["out"])  # (NG, 128, 512)
        e = blk.strides[2]
        v = np.lib.stride_tricks.as_strided(
            blk,
            shape=(NG, QB, NH, RW, 4),
            strides=(
                blk.strides[0],
                RW * blk.strides[1],
                NW * e,
                blk.strides[1] + 4 * e,
                e,
            ),
        )
        # [g, qi, h, r, j] -> b = QB*g + qi, s_rel = RW*h + r
        band[:, t * SB : (t + 1) * SB, :] = v.reshape(B, SB, 4)
    s_idx = np.arange(NT * SB)
    y = np.zeros((B, NT * SB, NT * NI), dtype=np.float32)
    y4 = y.reshape(B, NT * SB, NT * SB, 4)
    y4[:, s_idx, s_idx, :] = band
    return y
